# revision 4
# baseline (speedup 1.0000x reference)
"""GCNConv kernel for 8 Trainium2 NeuronCores (Bass/Tile).

Computes out = segment_sum(edge_val * (x @ W)[edge_col], edge_row) + b
as out = (A @ x) @ W + b  (associativity), with:
  - nodes (rows of output) sharded across 8 cores (12500 each)
  - edges partitioned by destination tile (128 rows), grouped 2 tiles per
    "group" and by source bank -> one dma_gather per (group, bank)
  - one-hot S matrices (S[e, dloc[e]] = val[e]): banks 0-1 use
    host-precomputed fp16 tiles DMA'd in; banks 2-3 are built on-chip by
    two wide DVE tensor_tensor ops using broadcast (stride-0) APs.
  - z[128 nodes, 256] += S_j.T @ X_block per block on the PE in PSUM.
  - epilogue per tile: transpose z, project by W (fp16), add bias, store.

x is split into 4 banks of 25000 rows because dma_gather indices are int16.
"""
import os
from contextlib import ExitStack

import numpy as np

import concourse.bass as bass
import concourse.tile as tile
from concourse import bacc, mybir
from concourse.bass_utils import run_bass_kernel_spmd

P = 128
D = 256
N_NODES = 100000
N_EDGES = 3200000
NC = 8
SH = N_NODES // NC          # 12500 rows per core
NT = (SH + P - 1) // P      # 98 tiles per core
GRP = 2                     # tiles per gather group
NG = NT // GRP              # 49 groups
SUP = 4                     # groups per idx/dv/sh load
NBANK = 4
NBH = 2                     # banks whose S comes precomputed from the host
BS = N_NODES // NBANK       # 25000 rows per bank (fits int16 index)
NCELL = NG * NBANK * GRP    # cells in block order (g, k, dt)

F16 = mybir.dt.float16
F32 = mybir.dt.float32
I16 = mybir.dt.int16

_last_results = None        # BassKernelResults of the most recent run


def _build_structure(edge_row, edge_col, edge_val):
    """Sort/pad edges into per-core 128-edge blocks ordered by
    (group of 2 dest tiles, source bank, dest tile).  Cell structure
    (nb_cell) is shared across cores (padded to the max) so one SPMD
    program fits all cores.

    Returns (nb_cell [NCELL] int, per-core dict arrays).
    """
    E = edge_row.shape[0]
    core = edge_row // SH
    r_loc = edge_row - core * SH
    t = r_loc // P
    dloc = (r_loc % P).astype(np.float16)
    g = t // GRP
    dt_ = t - g * GRP
    bank = edge_col // BS
    bidx = (edge_col % BS).astype(np.int16)

    cid = (g.astype(np.int64) * NBANK + bank) * GRP + dt_
    gid = core.astype(np.int64) * NCELL + cid
    order = np.argsort(gid, kind="stable")
    gid_s = gid[order]

    cnt = np.bincount(gid, minlength=NC * NCELL).reshape(NC, NCELL)
    nb_cell = (cnt.max(axis=0) + P - 1) // P        # [NCELL] blocks
    nb_cell = np.maximum(nb_cell, 1)                # keep structure non-empty
    NBLK = int(nb_cell.sum())
    pad_len = NBLK * P

    # slot offset of each cell within a core's padded edge list
    off_cell = np.zeros(NCELL, np.int64)
    flat_off = np.cumsum(nb_cell * P)
    off_cell[1:] = flat_off[:-1]

    # position of each edge within its (core, cell) run
    grp_start = np.zeros(E, np.int64)
    newgrp = np.ones(E, bool)
    newgrp[1:] = gid_s[1:] != gid_s[:-1]
    starts = np.where(newgrp)[0]
    grp_start[starts] = starts
    grp_start = np.maximum.accumulate(grp_start)
    pos_in_grp = np.arange(E) - grp_start

    cid_of_edge = gid_s % NCELL
    core_of_edge = gid_s // NCELL
    dest = off_cell[cid_of_edge] + pos_in_grp

    # per-block host-S mask (block-level, shared across cores)
    cell_bank = (np.arange(NCELL) // GRP) % NBANK
    blk_host = np.repeat(cell_bank < NBH, nb_cell)
    NBHOST = int(blk_host.sum())

    cores = []
    ev16 = edge_val.astype(np.float16)
    for c in range(NC):
        m = core_of_edge == c
        e_ids = order[m]
        d = dest[m]
        idx_arr = np.zeros(pad_len, np.int16)
        dloc_arr = np.zeros(pad_len, np.float16)
        val_arr = np.zeros(pad_len, np.float16)
        idx_arr[d] = bidx[e_ids]
        dloc_arr[d] = dloc[e_ids]
        val_arr[d] = ev16[e_ids]

        # packed gather indices: [128, 8*NBLK] int16 (16-wrap, replicated x8)
        idxp = np.tile(np.ascontiguousarray(idx_arr.reshape(-1, 16).T), (8, 1))
        dl = dloc_arr.reshape(NBLK, P)
        vv = val_arr.reshape(NBLK, P)
        # DVE-built blocks -> per-block [dloc, val]: [128, 2*NBD] f16
        dl16 = np.ascontiguousarray(dl[~blk_host].T)
        vv16 = np.ascontiguousarray(vv[~blk_host].T)
        nbd = dl16.shape[1]
        dv = np.empty((P, 2 * nbd), np.float16)
        dv[:, 0::2] = dl16
        dv[:, 1::2] = vv16
        # host blocks -> dense one-hot S: [128, NBHOST*128] f16
        dlh = dl[blk_host].astype(np.int64)     # [NBHOST, 128e]
        vvh = vv[blk_host]                      # [NBHOST, 128e]
        shm = np.zeros((NBHOST, P, P), np.float16)  # [j, e, d]
        np.put_along_axis(shm, dlh[:, :, None], vvh[:, :, None], axis=2)
        sh = np.ascontiguousarray(shm.transpose(1, 0, 2).reshape(P, NBHOST * P))
        cores.append(dict(idxp=idxp, dv=dv, sh=sh))

    return nb_cell, cores


def _build_program(nb_cell):
    """Build the SPMD Bass program for the given cell structure."""
    cells = np.asarray(nb_cell).reshape(NG, NBANK, GRP)
    nb_g = cells.sum(axis=(1, 2))                   # [NG] blocks per group
    nb_gh = cells[:, :NBH, :].sum(axis=(1, 2))      # [NG] host-S blocks
    nb_gd = cells[:, NBH:, :].sum(axis=(1, 2))      # [NG] DVE-S blocks
    NBLK = int(nb_g.sum())
    NBHOST = int(nb_gh.sum())
    NBD = int(nb_gd.sum())
    gnb_max = int(nb_g.max())
    gd_max = int(nb_gd.max())
    sn_all = [int(nb_g[s:s + SUP].sum()) for s in range(0, NG, SUP)]
    snh = [int(nb_gh[s:s + SUP].sum()) for s in range(0, NG, SUP)]
    snd = [int(nb_gd[s:s + SUP].sum()) for s in range(0, NG, SUP)]
    out_rows = NT * P

    nc = bacc.Bacc("TRN2", target_bir_lowering=False, debug=False,
                   num_devices=NC, num_swdge_queues=4)
    xb_aps = [nc.dram_tensor(f"xb{k}", [BS, D], F16,
                             kind="ExternalInput").ap() for k in range(NBANK)]
    idxp_ap = nc.dram_tensor("idxp", [P, 8 * NBLK], I16,
                             kind="ExternalInput").ap()
    dv_ap = nc.dram_tensor("dv", [P, 2 * NBD], F16,
                           kind="ExternalInput").ap()
    sh_ap = nc.dram_tensor("sh", [P, NBHOST * P], F16,
                           kind="ExternalInput").ap()
    w_ap = nc.dram_tensor("w", [D, D], F16, kind="ExternalInput").ap()
    bias_ap = nc.dram_tensor("bias", [P, D], F32, kind="ExternalInput").ap()
    iota_ap = nc.dram_tensor("iota", [P, P], F16, kind="ExternalInput").ap()
    ident_ap = nc.dram_tensor("ident", [P, P], F16, kind="ExternalInput").ap()
    out_ap = nc.dram_tensor("out", [out_rows, D], F32,
                            kind="ExternalOutput").ap()

    sa_max, sh_max, sd_max = max(sn_all), max(snh), max(snd)

    with tile.TileContext(nc) as tc:
        with ExitStack() as ctx:
            const = ctx.enter_context(tc.tile_pool(name="const", bufs=1))
            idxpool = ctx.enter_context(tc.tile_pool(name="idxp", bufs=2))
            dvpool = ctx.enter_context(tc.tile_pool(name="dvp", bufs=2))
            shpool = ctx.enter_context(tc.tile_pool(name="shp", bufs=2))
            xgpool = ctx.enter_context(tc.tile_pool(name="xgp", bufs=2))
            swpool = ctx.enter_context(tc.tile_pool(name="swp", bufs=2))
            epool = ctx.enter_context(tc.tile_pool(name="ep", bufs=2))
            zpsum = ctx.enter_context(
                tc.tile_pool(name="zps", bufs=4, space="PSUM"))
            tpsum = ctx.enter_context(
                tc.tile_pool(name="tps", bufs=2, space="PSUM"))
            opsum = ctx.enter_context(
                tc.tile_pool(name="ops", bufs=2, space="PSUM"))

            iota_t = const.tile([P, P], F16, tag="iota")
            nc.sync.dma_start(iota_t[:], iota_ap[:])
            ident_t = const.tile([P, P], F16, tag="ident")
            nc.sync.dma_start(ident_t[:], ident_ap[:])
            w_t = const.tile([P, 2, D], F16, tag="w")
            nc.sync.dma_start(w_t[:], w_ap[:].rearrange("(c k) d -> k c d",
                                                        k=P))
            bias_t = const.tile([P, D], F32, tag="bias")
            nc.sync.dma_start(bias_t[:], bias_ap[:])

            bo = boh = bod = 0          # global block offsets (all/host/dve)
            sbo = sboh = sbod = 0       # offsets at current super start
            idx_t = dv_t = sh_t = None
            for g in range(NG):
                if g % SUP == 0:
                    s = g // SUP
                    sbo, sboh, sbod = bo, boh, bod
                    idx_t = idxpool.tile([P, 8 * sa_max], I16, tag="idx")
                    nc.sync.dma_start(
                        idx_t[:, :8 * sn_all[s]],
                        idxp_ap[:, 8 * bo:8 * (bo + sn_all[s])])
                    dv_t = dvpool.tile([P, sd_max, 2], F16, tag="dv")
                    nc.sync.dma_start(
                        dv_t[:, :snd[s], :],
                        dv_ap[:, 2 * bod:2 * (bod + snd[s])].rearrange(
                            "p (n two) -> p n two", two=2))
                    sh_t = shpool.tile([P, sh_max, P], F16, tag="sh")
                    nc.sync.dma_start(
                        sh_t[:, :snh[s], :],
                        sh_ap[:, P * boh:P * (boh + snh[s])].rearrange(
                            "p (n q) -> p n q", q=P))
                lo = bo - sbo
                loh = boh - sboh
                lod = bod - sbod
                gnbd = int(nb_gd[g])

                xg = xgpool.tile([P, gnb_max, D], F16, tag="xg")
                ok = 0
                okk = []
                for k in range(NBANK):
                    nbk = int(cells[g, k, :].sum())
                    okk.append(ok)
                    n = nbk * P
                    nc.gpsimd.dma_gather(
                        out_ap=xg[:, ok:ok + nbk, :],
                        in_ap=xb_aps[k][:],
                        idxs_ap=idx_t[:, 8 * (lo + ok):8 * (lo + ok + nbk)],
                        num_idxs=n,
                        num_idxs_reg=n,
                        elem_size=D,
                        single_packet=(n <= 992),
                        queue_num=k,
                    )
                    ok += nbk

                sw = swpool.tile([P, gd_max, P], F16, tag="sw")
                dloc_b = dv_t[:, lod:lod + gnbd, 0:1].broadcast_to(
                    (P, gnbd, P))
                val_b = dv_t[:, lod:lod + gnbd, 1:2].broadcast_to(
                    (P, gnbd, P))
                iota_b = iota_t[:].unsqueeze(1).broadcast_to((P, gnbd, P))
                nc.vector.tensor_tensor(out=sw[:, :gnbd, :], in0=iota_b,
                                        in1=dloc_b,
                                        op=mybir.AluOpType.is_equal)
                nc.vector.tensor_tensor(out=sw[:, :gnbd, :],
                                        in0=sw[:, :gnbd, :],
                                        in1=val_b, op=mybir.AluOpType.mult)

                # in-group offsets of the host-S / DVE-S compact streams
                hoff = [0]
                doff = [0]
                for k in range(NBANK):
                    nbk = int(cells[g, k, :].sum())
                    if k < NBH:
                        hoff.append(hoff[-1] + nbk)
                    else:
                        doff.append(doff[-1] + nbk)
                for dt_i in range(GRP):
                    t = g * GRP + dt_i
                    mms = []  # (lhsT ap, xg block position)
                    for k in range(NBANK):
                        seg = okk[k] + (int(cells[g, k, 0]) if dt_i else 0)
                        if k < NBH:
                            sseg = hoff[k] + (int(cells[g, k, 0])
                                              if dt_i else 0)
                            for i2, jj in enumerate(
                                    range(seg, seg + int(cells[g, k, dt_i]))):
                                mms.append((sh_t[:, loh + sseg + i2, :], jj))
                        else:
                            sseg = doff[k - NBH] + (int(cells[g, k, 0])
                                                    if dt_i else 0)
                            for i2, jj in enumerate(
                                    range(seg, seg + int(cells[g, k, dt_i]))):
                                mms.append((sw[:, sseg + i2, :], jj))
                    z_ps = zpsum.tile([P, D], F32, tag="zps")
                    for i, (s_ap, jj) in enumerate(mms):
                        nc.tensor.matmul(out=z_ps[:], lhsT=s_ap,
                                         rhs=xg[:, jj, :],
                                         start=(i == 0),
                                         stop=(i == len(mms) - 1))

                    z_sb = epool.tile([P, D], F16, tag="zsb")
                    nc.scalar.copy(z_sb[:], z_ps[:])
                    o_ps = opsum.tile([P, D], F32, tag="ops")
                    for ch in range(2):
                        zt_ps = tpsum.tile([P, P], F16, tag="ztps")
                        nc.tensor.transpose(zt_ps[:],
                                            z_sb[:, ch * P:(ch + 1) * P],
                                            ident_t[:])
                        zt_sb = epool.tile([P, P], F16, tag="ztsb")
                        nc.scalar.copy(zt_sb[:], zt_ps[:])
                        nc.tensor.matmul(out=o_ps[:], lhsT=zt_sb[:],
                                         rhs=w_t[:, ch, :],
                                         start=(ch == 0), stop=(ch == 1))
                    o_sb = epool.tile([P, D], F32, tag="osb")
                    nc.vector.tensor_add(o_sb[:], o_ps[:], bias_t[:])
                    nc.sync.dma_start(out_ap[t * P:(t + 1) * P, :], o_sb[:])
                bo += int(nb_g[g])
                boh += int(nb_gh[g])
                bod += gnbd
    nc.compile()
    return nc


def kernel(x, edge_row, edge_col, edge_val, weight, b):
    global _last_results
    assert x.shape == (N_NODES, D)

    nb_cell, cores = _build_structure(
        np.asarray(edge_row), np.asarray(edge_col), np.asarray(edge_val))
    nc = _build_program(nb_cell)

    x16 = np.asarray(x, np.float32).astype(np.float16)
    banks = [np.ascontiguousarray(x16[k * BS:(k + 1) * BS])
             for k in range(NBANK)]
    w16 = np.asarray(weight, np.float32).astype(np.float16)
    bias = np.broadcast_to(
        np.asarray(b, np.float32)[None, :], (P, D)).copy()
    iota = np.tile(np.arange(P, dtype=np.float16)[None, :], (P, 1))
    ident = np.eye(P, dtype=np.float16)

    in_maps = []
    for c in range(NC):
        m = {f"xb{k}": banks[k] for k in range(NBANK)}
        m.update(idxp=cores[c]["idxp"], dv=cores[c]["dv"], sh=cores[c]["sh"],
                 w=w16, bias=bias, iota=iota, ident=ident)
        in_maps.append(m)

    trace = bool(os.environ.get("KERNEL_TRACE"))
    res = run_bass_kernel_spmd(nc, in_maps, list(range(NC)), trace=trace)
    _last_results = res

    out = np.concatenate([res.results[c]["out"][:SH] for c in range(NC)],
                         axis=0)
    return out.astype(np.float32)


# revision 6
# speedup vs baseline: 1.1899x; 1.1899x over previous
"""GCNConv kernel for 8 Trainium2 NeuronCores (Bass/Tile).

Computes out = segment_sum(edge_val * (x @ W)[edge_col], edge_row) + b
as out = (A @ x) @ W + b  (associativity), with:
  - nodes (rows of output) sharded across 8 cores (12500 each)
  - edges partitioned by destination tile (128 rows), grouped 2 tiles per
    "group" and by source bank -> one dma_gather per (group, bank);
    edges are sorted by source index within each cell so the gather's
    HBM reads are monotone (row-buffer locality)
  - S one-hot matrices (S[e, dloc[e]] = val[e]) for a whole group are built
    with TWO wide DVE tensor_tensor ops using broadcast (stride-0) APs:
      sw = is_equal(iota_bcast, dloc_bcast); sw *= val_bcast
  - z[128 nodes, 256] += S_j.T @ X_block per block on the PE in PSUM.
  - epilogue per tile: transpose z, project by W (fp16), add bias, store.

x is split into 4 banks of 25000 rows because dma_gather indices are int16.
"""
import os
from contextlib import ExitStack

import numpy as np

import concourse.bass as bass
import concourse.tile as tile
from concourse import bacc, mybir
from concourse.bass_utils import run_bass_kernel_spmd

P = 128
D = 256
N_NODES = 100000
N_EDGES = 3200000
NC = 8
SH = N_NODES // NC          # 12500 rows per core
NT = (SH + P - 1) // P      # 98 tiles per core
GRP = 2                     # tiles per gather group
NG = NT // GRP              # 49 groups
SUP = 4                     # groups per idx/dv load
NBANK = 4
BS = N_NODES // NBANK       # 25000 rows per bank (fits int16 index)
NCELL = NG * NBANK * GRP    # cells in block order (g, k, dt)

F16 = mybir.dt.float16
F32 = mybir.dt.float32
I16 = mybir.dt.int16

_last_results = None        # BassKernelResults of the most recent run


def _build_structure(edge_row, edge_col, edge_val):
    """Sort/pad edges into per-core 128-edge blocks ordered by
    (group of 2 dest tiles, source bank, dest tile), sorted by source
    index within each cell.  Cell structure (nb_cell) is shared across
    cores (padded to the max) so one SPMD program fits all cores.

    Returns (nb_cell [NCELL] int, per-core dict arrays).
    """
    E = edge_row.shape[0]
    core = edge_row // SH
    r_loc = edge_row - core * SH
    t = r_loc // P
    dloc = (r_loc % P).astype(np.float16)
    g = t // GRP
    dt_ = t - g * GRP
    bank = edge_col // BS
    bidx = (edge_col % BS).astype(np.int16)

    cid = (g.astype(np.int64) * NBANK + bank) * GRP + dt_
    gid = core.astype(np.int64) * NCELL + cid
    # sort by (core, cell, src index) -> monotone HBM reads per gather
    order = np.argsort(gid * (BS + 1) + bidx, kind="stable")
    gid_s = gid[order]

    cnt = np.bincount(gid, minlength=NC * NCELL).reshape(NC, NCELL)
    nb_cell = (cnt.max(axis=0) + P - 1) // P        # [NCELL] blocks
    nb_cell = np.maximum(nb_cell, 1)                # keep structure non-empty
    NBLK = int(nb_cell.sum())
    pad_len = NBLK * P

    # slot offset of each cell within a core's padded edge list
    off_cell = np.zeros(NCELL, np.int64)
    flat_off = np.cumsum(nb_cell * P)
    off_cell[1:] = flat_off[:-1]

    # position of each edge within its (core, cell) run
    grp_start = np.zeros(E, np.int64)
    newgrp = np.ones(E, bool)
    newgrp[1:] = gid_s[1:] != gid_s[:-1]
    starts = np.where(newgrp)[0]
    grp_start[starts] = starts
    grp_start = np.maximum.accumulate(grp_start)
    pos_in_grp = np.arange(E) - grp_start

    cid_of_edge = gid_s % NCELL
    core_of_edge = gid_s // NCELL
    dest = off_cell[cid_of_edge] + pos_in_grp

    cores = []
    ev16 = edge_val.astype(np.float16)
    for c in range(NC):
        m = core_of_edge == c
        e_ids = order[m]
        d = dest[m]
        idx_arr = np.zeros(pad_len, np.int16)
        dloc_arr = np.zeros(pad_len, np.float16)
        val_arr = np.zeros(pad_len, np.float16)
        idx_arr[d] = bidx[e_ids]
        dloc_arr[d] = dloc[e_ids]
        val_arr[d] = ev16[e_ids]

        # packed gather indices: [128, 8*NBLK] int16 (16-wrap, replicated x8)
        idxp = np.tile(np.ascontiguousarray(idx_arr.reshape(-1, 16).T), (8, 1))
        # per-block [dloc, val]: [128, 2*NBLK] f16
        dl = np.ascontiguousarray(dloc_arr.reshape(NBLK, P).T)
        vv = np.ascontiguousarray(val_arr.reshape(NBLK, P).T)
        dv = np.empty((P, 2 * NBLK), np.float16)
        dv[:, 0::2] = dl
        dv[:, 1::2] = vv
        cores.append(dict(idxp=idxp, dv=dv))

    return nb_cell, cores


def _build_program(nb_cell):
    """Build the SPMD Bass program for the given cell structure."""
    cells = np.asarray(nb_cell).reshape(NG, NBANK, GRP)
    nb_g = cells.sum(axis=(1, 2))                   # [NG] blocks per group
    NBLK = int(nb_g.sum())
    gnb_max = int(nb_g.max())
    snb = [int(nb_g[s:s + SUP].sum()) for s in range(0, NG, SUP)]
    snb_max = max(snb)
    out_rows = NT * P

    nc = bacc.Bacc("TRN2", target_bir_lowering=False, debug=False,
                   num_devices=NC, num_swdge_queues=4,
                   dynamic_dma_scratch_size=32768)
    xb_aps = [nc.dram_tensor(f"xb{k}", [BS, D], F16,
                             kind="ExternalInput").ap() for k in range(NBANK)]
    idxp_ap = nc.dram_tensor("idxp", [P, 8 * NBLK], I16,
                             kind="ExternalInput").ap()
    dv_ap = nc.dram_tensor("dv", [P, 2 * NBLK], F16,
                           kind="ExternalInput").ap()
    w_ap = nc.dram_tensor("w", [D, D], F16, kind="ExternalInput").ap()
    bias_ap = nc.dram_tensor("bias", [P, D], F32, kind="ExternalInput").ap()
    iota_ap = nc.dram_tensor("iota", [P, P], F16, kind="ExternalInput").ap()
    ident_ap = nc.dram_tensor("ident", [P, P], F16, kind="ExternalInput").ap()
    out_ap = nc.dram_tensor("out", [out_rows, D], F32,
                            kind="ExternalOutput").ap()

    with tile.TileContext(nc) as tc:
        with ExitStack() as ctx:
            const = ctx.enter_context(tc.tile_pool(name="const", bufs=1))
            idxpool = ctx.enter_context(tc.tile_pool(name="idxp", bufs=2))
            dvpool = ctx.enter_context(tc.tile_pool(name="dvp", bufs=2))
            xgpool = ctx.enter_context(tc.tile_pool(name="xgp", bufs=3))
            swpool = ctx.enter_context(tc.tile_pool(name="swp", bufs=2))
            epool = ctx.enter_context(tc.tile_pool(name="ep", bufs=2))
            zpsum = ctx.enter_context(
                tc.tile_pool(name="zps", bufs=4, space="PSUM"))
            tpsum = ctx.enter_context(
                tc.tile_pool(name="tps", bufs=2, space="PSUM"))
            opsum = ctx.enter_context(
                tc.tile_pool(name="ops", bufs=2, space="PSUM"))

            iota_t = const.tile([P, P], F16, tag="iota")
            nc.sync.dma_start(iota_t[:], iota_ap[:])
            ident_t = const.tile([P, P], F16, tag="ident")
            nc.sync.dma_start(ident_t[:], ident_ap[:])
            w_t = const.tile([P, 2, D], F16, tag="w")
            nc.sync.dma_start(w_t[:], w_ap[:].rearrange("(c k) d -> k c d",
                                                        k=P))
            bias_t = const.tile([P, D], F32, tag="bias")
            nc.sync.dma_start(bias_t[:], bias_ap[:])

            bo = 0          # global block offset
            sbo = 0         # block offset of current super-group start
            idx_t = dv_t = None
            for g in range(NG):
                if g % SUP == 0:
                    sn = snb[g // SUP]
                    sbo = bo
                    idx_t = idxpool.tile([P, 8 * snb_max], I16, tag="idx")
                    nc.sync.dma_start(idx_t[:, :8 * sn],
                                      idxp_ap[:, 8 * bo:8 * (bo + sn)])
                    dv_t = dvpool.tile([P, snb_max, 2], F16, tag="dv")
                    nc.sync.dma_start(
                        dv_t[:, :sn, :],
                        dv_ap[:, 2 * bo:2 * (bo + sn)].rearrange(
                            "p (n two) -> p n two", two=2))
                lo = bo - sbo   # group's block offset within the super tiles
                gnb = int(nb_g[g])
                xg = xgpool.tile([P, gnb_max, D], F16, tag="xg")
                ok = 0
                for k in range(NBANK):
                    nbk = int(cells[g, k, :].sum())
                    n = nbk * P
                    nc.gpsimd.dma_gather(
                        out_ap=xg[:, ok:ok + nbk, :],
                        in_ap=xb_aps[k][:],
                        idxs_ap=idx_t[:, 8 * (lo + ok):8 * (lo + ok + nbk)],
                        num_idxs=n,
                        num_idxs_reg=n,
                        elem_size=D,
                        single_packet=(n <= 992),
                        queue_num=k,
                    )
                    ok += nbk

                sw = swpool.tile([P, gnb_max, P], F16, tag="sw")
                dloc_b = dv_t[:, lo:lo + gnb, 0:1].broadcast_to((P, gnb, P))
                val_b = dv_t[:, lo:lo + gnb, 1:2].broadcast_to((P, gnb, P))
                iota_b = iota_t[:].unsqueeze(1).broadcast_to((P, gnb, P))
                nc.vector.tensor_tensor(out=sw[:, :gnb, :], in0=iota_b,
                                        in1=dloc_b,
                                        op=mybir.AluOpType.is_equal)
                nc.vector.tensor_tensor(out=sw[:, :gnb, :], in0=sw[:, :gnb, :],
                                        in1=val_b, op=mybir.AluOpType.mult)

                okk = np.concatenate(
                    [[0], np.cumsum(cells[g].sum(axis=1))])  # [NBANK+1]
                for dt_i in range(GRP):
                    t = g * GRP + dt_i
                    blist = []
                    for k in range(NBANK):
                        seg = int(okk[k]) + (int(cells[g, k, 0]) if dt_i else 0)
                        blist.extend(range(seg, seg + int(cells[g, k, dt_i])))
                    z_ps = zpsum.tile([P, D], F32, tag="zps")
                    for i, jj in enumerate(blist):
                        nc.tensor.matmul(out=z_ps[:], lhsT=sw[:, jj, :],
                                         rhs=xg[:, jj, :],
                                         start=(i == 0),
                                         stop=(i == len(blist) - 1))

                    z_sb = epool.tile([P, D], F16, tag="zsb")
                    nc.scalar.copy(z_sb[:], z_ps[:])
                    o_ps = opsum.tile([P, D], F32, tag="ops")
                    for ch in range(2):
                        zt_ps = tpsum.tile([P, P], F16, tag="ztps")
                        nc.tensor.transpose(zt_ps[:],
                                            z_sb[:, ch * P:(ch + 1) * P],
                                            ident_t[:])
                        zt_sb = epool.tile([P, P], F16, tag="ztsb")
                        nc.scalar.copy(zt_sb[:], zt_ps[:])
                        nc.tensor.matmul(out=o_ps[:], lhsT=zt_sb[:],
                                         rhs=w_t[:, ch, :],
                                         start=(ch == 0), stop=(ch == 1))
                    o_sb = epool.tile([P, D], F32, tag="osb")
                    nc.vector.tensor_add(o_sb[:], o_ps[:], bias_t[:])
                    nc.sync.dma_start(out_ap[t * P:(t + 1) * P, :], o_sb[:])
                bo += gnb
    nc.compile()
    return nc


def kernel(x, edge_row, edge_col, edge_val, weight, b):
    global _last_results
    assert x.shape == (N_NODES, D)

    nb_cell, cores = _build_structure(
        np.asarray(edge_row), np.asarray(edge_col), np.asarray(edge_val))
    nc = _build_program(nb_cell)

    x16 = np.asarray(x, np.float32).astype(np.float16)
    banks = [np.ascontiguousarray(x16[k * BS:(k + 1) * BS])
             for k in range(NBANK)]
    w16 = np.asarray(weight, np.float32).astype(np.float16)
    bias = np.broadcast_to(
        np.asarray(b, np.float32)[None, :], (P, D)).copy()
    iota = np.tile(np.arange(P, dtype=np.float16)[None, :], (P, 1))
    ident = np.eye(P, dtype=np.float16)

    in_maps = []
    for c in range(NC):
        m = {f"xb{k}": banks[k] for k in range(NBANK)}
        m.update(idxp=cores[c]["idxp"], dv=cores[c]["dv"], w=w16,
                 bias=bias, iota=iota, ident=ident)
        in_maps.append(m)

    trace = bool(os.environ.get("KERNEL_TRACE"))
    res = run_bass_kernel_spmd(nc, in_maps, list(range(NC)), trace=trace)
    _last_results = res

    out = np.concatenate([res.results[c]["out"][:SH] for c in range(NC)],
                         axis=0)
    return out.astype(np.float32)


# revision 17
# speedup vs baseline: 1.2129x; 1.0193x over previous
"""GCNConv kernel for 8 Trainium2 NeuronCores (Bass/Tile).

Computes out = segment_sum(edge_val * (x @ W)[edge_col], edge_row) + b
as out = (A @ x) @ W + b  (associativity), with:
  - nodes (rows of output) sharded across 8 cores (12500 each)
  - edges partitioned by destination tile (128 rows), grouped 2 tiles per
    "group" and by source bank -> one dma_gather per (group, bank);
    edges are sorted by source index within each cell so the gather's
    HBM reads are monotone (row-buffer locality)
  - S one-hot matrices (S[e, dloc[e]] = val[e]) for a whole group are built
    with TWO wide DVE tensor_tensor ops using broadcast (stride-0) APs:
      sw = is_equal(iota_bcast, dloc_bcast); sw *= val_bcast
  - z[128 nodes, 256] += S_j.T @ X_block per block on the PE in PSUM.
  - epilogue per tile: transpose z, project by W (fp16), add bias, store.

x is split into 4 banks of 25000 rows because dma_gather indices are int16.
"""
import os
from contextlib import ExitStack

import numpy as np

import concourse.bass as bass
import concourse.tile as tile
from concourse import bacc, mybir
from concourse.bass_utils import run_bass_kernel_spmd

P = 128
D = 256
N_NODES = 100000
N_EDGES = 3200000
NC = 8
SH = N_NODES // NC          # 12500 rows per core
NT = (SH + P - 1) // P      # 98 tiles per core
GRP = 2                     # tiles per gather group
NG = NT // GRP              # 49 groups
SUP = 4                     # groups per idx/dv load
NBANK = 4
BS = N_NODES // NBANK       # 25000 rows per bank (fits int16 index)
NCELL = NG * NBANK * GRP    # cells in block order (g, k, dt)
HOSTK = 3                   # bank whose S comes precomputed from the host
SUPH = 2                    # groups per host-S load

F16 = mybir.dt.float16
F32 = mybir.dt.float32
I16 = mybir.dt.int16

_last_results = None        # BassKernelResults of the most recent run


def _build_structure(edge_row, edge_col, edge_val):
    """Sort/pad edges into per-core 128-edge blocks ordered by
    (group of 2 dest tiles, source bank, dest tile), sorted by source
    index within each cell.  Cell structure (nb_cell) is shared across
    cores (padded to the max) so one SPMD program fits all cores.

    Returns (nb_cell [NCELL] int, per-core dict arrays).
    """
    E = edge_row.shape[0]
    core = edge_row // SH
    r_loc = edge_row - core * SH
    t = r_loc // P
    dloc = (r_loc % P).astype(np.float16)
    g = t // GRP
    dt_ = t - g * GRP
    bank = edge_col // BS
    bidx = (edge_col % BS).astype(np.int16)

    cid = (g.astype(np.int64) * NBANK + bank) * GRP + dt_
    gid = core.astype(np.int64) * NCELL + cid
    # sort by (core, cell, src index) -> monotone HBM reads per gather
    order = np.argsort(gid * (BS + 1) + bidx, kind="stable")
    gid_s = gid[order]

    cnt = np.bincount(gid, minlength=NC * NCELL).reshape(NC, NCELL)
    nb_cell = (cnt.max(axis=0) + P - 1) // P        # [NCELL] blocks
    nb_cell = np.maximum(nb_cell, 1)                # keep structure non-empty
    NBLK = int(nb_cell.sum())
    pad_len = NBLK * P

    # slot offset of each cell within a core's padded edge list
    off_cell = np.zeros(NCELL, np.int64)
    flat_off = np.cumsum(nb_cell * P)
    off_cell[1:] = flat_off[:-1]

    # position of each edge within its (core, cell) run
    grp_start = np.zeros(E, np.int64)
    newgrp = np.ones(E, bool)
    newgrp[1:] = gid_s[1:] != gid_s[:-1]
    starts = np.where(newgrp)[0]
    grp_start[starts] = starts
    grp_start = np.maximum.accumulate(grp_start)
    pos_in_grp = np.arange(E) - grp_start

    cid_of_edge = gid_s % NCELL
    core_of_edge = gid_s // NCELL
    dest = off_cell[cid_of_edge] + pos_in_grp

    # per-block host-S mask: bank HOSTK blocks get host-precomputed S
    cell_bank = (np.arange(NCELL) // GRP) % NBANK
    blk_host = np.repeat(cell_bank == HOSTK, nb_cell)
    NBHOST = int(blk_host.sum())

    cores = []
    ev16 = edge_val.astype(np.float16)
    for c in range(NC):
        m = core_of_edge == c
        e_ids = order[m]
        d = dest[m]
        idx_arr = np.zeros(pad_len, np.int16)
        dloc_arr = np.zeros(pad_len, np.float16)
        val_arr = np.zeros(pad_len, np.float16)
        idx_arr[d] = bidx[e_ids]
        dloc_arr[d] = dloc[e_ids]
        val_arr[d] = ev16[e_ids]

        # packed gather indices: [128, 8*NBLK] int16 (16-wrap, replicated x8)
        idxp = np.tile(np.ascontiguousarray(idx_arr.reshape(-1, 16).T), (8, 1))
        dl = dloc_arr.reshape(NBLK, P)
        vv = val_arr.reshape(NBLK, P)
        # DVE-built blocks -> per-block [dloc, val]: [128, 2*NBD] f16
        dl16 = np.ascontiguousarray(dl[~blk_host].T)
        vv16 = np.ascontiguousarray(vv[~blk_host].T)
        nbd = dl16.shape[1]
        dv = np.empty((P, 2 * nbd), np.float16)
        dv[:, 0::2] = dl16
        dv[:, 1::2] = vv16
        # host blocks -> dense one-hot S: [128, NBHOST*128] f16
        dlh = dl[blk_host].astype(np.int64)
        vvh = vv[blk_host]
        shm = np.zeros((NBHOST, P, P), np.float16)
        np.put_along_axis(shm, dlh[:, :, None], vvh[:, :, None], axis=2)
        sh = np.ascontiguousarray(shm.transpose(1, 0, 2).reshape(P, NBHOST * P))
        cores.append(dict(idxp=idxp, dv=dv, sh=sh))

    return nb_cell, cores


def _build_program(nb_cell):
    """Build the SPMD Bass program for the given cell structure."""
    cells = np.asarray(nb_cell).reshape(NG, NBANK, GRP)
    nb_g = cells.sum(axis=(1, 2))                   # [NG] blocks per group
    nb_gh = cells[:, HOSTK, :].sum(axis=1)          # [NG] host-S blocks
    nb_gd = nb_g - nb_gh                            # [NG] DVE-S blocks
    NBLK = int(nb_g.sum())
    NBHOST = int(nb_gh.sum())
    NBD = int(nb_gd.sum())
    gnb_max = int(nb_g.max())
    gd_max = int(nb_gd.max())
    snb = [int(nb_g[s:s + SUP].sum()) for s in range(0, NG, SUP)]
    snd = [int(nb_gd[s:s + SUP].sum()) for s in range(0, NG, SUP)]
    snb_max = max(snb)
    snd_max = max(snd)
    snh = [int(nb_gh[s:s + SUPH].sum()) for s in range(0, NG, SUPH)]
    snh_max = max(snh)
    out_rows = NT * P

    nc = bacc.Bacc("TRN2", target_bir_lowering=False, debug=False,
                   num_devices=NC, num_swdge_queues=4,
                   dynamic_dma_scratch_size=32768)
    xb_aps = [nc.dram_tensor(f"xb{k}", [BS, D], F16,
                             kind="ExternalInput").ap() for k in range(NBANK)]
    idxp_ap = nc.dram_tensor("idxp", [P, 8 * NBLK], I16,
                             kind="ExternalInput").ap()
    dv_ap = nc.dram_tensor("dv", [P, 2 * NBD], F16,
                           kind="ExternalInput").ap()
    sh_ap = nc.dram_tensor("sh", [P, NBHOST * P], F16,
                           kind="ExternalInput").ap()
    w_ap = nc.dram_tensor("w", [D, D], F16, kind="ExternalInput").ap()
    bias_ap = nc.dram_tensor("bias", [P, D], F32, kind="ExternalInput").ap()
    iota_ap = nc.dram_tensor("iota", [P, P], F16, kind="ExternalInput").ap()
    ident_ap = nc.dram_tensor("ident", [P, P], F16, kind="ExternalInput").ap()
    out_ap = nc.dram_tensor("out", [out_rows, D], F32,
                            kind="ExternalOutput").ap()

    with tile.TileContext(nc) as tc:
        with ExitStack() as ctx:
            const = ctx.enter_context(tc.tile_pool(name="const", bufs=1))
            idxpool = ctx.enter_context(tc.tile_pool(name="idxp", bufs=2))
            dvpool = ctx.enter_context(tc.tile_pool(name="dvp", bufs=2))
            shpool = ctx.enter_context(tc.tile_pool(name="shp", bufs=2))
            xgpool = ctx.enter_context(tc.tile_pool(name="xgp", bufs=3))
            swpool = ctx.enter_context(tc.tile_pool(name="swp", bufs=2))
            epool = ctx.enter_context(tc.tile_pool(name="ep", bufs=2))
            zpsum = ctx.enter_context(
                tc.tile_pool(name="zps", bufs=4, space="PSUM"))
            tpsum = ctx.enter_context(
                tc.tile_pool(name="tps", bufs=2, space="PSUM"))
            opsum = ctx.enter_context(
                tc.tile_pool(name="ops", bufs=2, space="PSUM"))

            iota_t = const.tile([P, P], F16, tag="iota")
            nc.sync.dma_start(iota_t[:], iota_ap[:])
            ident_t = const.tile([P, P], F16, tag="ident")
            nc.sync.dma_start(ident_t[:], ident_ap[:])
            w_t = const.tile([P, 2, D], F16, tag="w")
            nc.sync.dma_start(w_t[:], w_ap[:].rearrange("(c k) d -> k c d",
                                                        k=P))
            bias_t = const.tile([P, D], F32, tag="bias")
            nc.sync.dma_start(bias_t[:], bias_ap[:])

            bo = bod = boh = 0   # global block offsets (all/dve/host)
            sbo = sbod = sboh = 0  # offsets at current super starts
            idx_t = dv_t = sh_t = None
            for g in range(NG):
                if g % SUP == 0:
                    s = g // SUP
                    sn = snb[s]
                    sbo, sbod = bo, bod
                    idx_t = idxpool.tile([P, 8 * snb_max], I16, tag="idx")
                    nc.sync.dma_start(idx_t[:, :8 * sn],
                                      idxp_ap[:, 8 * bo:8 * (bo + sn)])
                    dv_t = dvpool.tile([P, snd_max, 2], F16, tag="dv")
                    nc.sync.dma_start(
                        dv_t[:, :snd[s], :],
                        dv_ap[:, 2 * bod:2 * (bod + snd[s])].rearrange(
                            "p (n two) -> p n two", two=2))
                if g % SUPH == 0:
                    sh2 = g // SUPH
                    sboh = boh
                    sh_t = shpool.tile([P, snh_max, P], F16, tag="sh")
                    nc.sync.dma_start(
                        sh_t[:, :snh[sh2], :],
                        sh_ap[:, P * boh:P * (boh + snh[sh2])].rearrange(
                            "p (n q) -> p n q", q=P))
                lo = bo - sbo   # group's block offset within the super tiles
                lod = bod - sbod
                loh = boh - sboh
                gnb = int(nb_g[g])
                gnbd = int(nb_gd[g])
                xg = xgpool.tile([P, gnb_max, D], F16, tag="xg")
                ok = 0
                for k in range(NBANK):
                    nbk = int(cells[g, k, :].sum())
                    n = nbk * P
                    nc.gpsimd.dma_gather(
                        out_ap=xg[:, ok:ok + nbk, :],
                        in_ap=xb_aps[k][:],
                        idxs_ap=idx_t[:, 8 * (lo + ok):8 * (lo + ok + nbk)],
                        num_idxs=n,
                        num_idxs_reg=n,
                        elem_size=D,
                        single_packet=(n <= 992),
                        queue_num=k,
                    )
                    ok += nbk

                sw = swpool.tile([P, gd_max, P], F16, tag="sw")
                dloc_b = dv_t[:, lod:lod + gnbd, 0:1].broadcast_to(
                    (P, gnbd, P))
                val_b = dv_t[:, lod:lod + gnbd, 1:2].broadcast_to(
                    (P, gnbd, P))
                iota_b = iota_t[:].unsqueeze(1).broadcast_to((P, gnbd, P))
                nc.vector.tensor_tensor(out=sw[:, :gnbd, :], in0=iota_b,
                                        in1=dloc_b,
                                        op=mybir.AluOpType.is_equal)
                nc.vector.tensor_tensor(out=sw[:, :gnbd, :],
                                        in0=sw[:, :gnbd, :],
                                        in1=val_b, op=mybir.AluOpType.mult)

                okk = np.concatenate(
                    [[0], np.cumsum(cells[g].sum(axis=1))])  # [NBANK+1]
                for dt_i in range(GRP):
                    t = g * GRP + dt_i
                    blist = []
                    for k in range(NBANK):
                        seg = int(okk[k]) + (int(cells[g, k, 0]) if dt_i else 0)
                        blist.extend(range(seg, seg + int(cells[g, k, dt_i])))
                    z_ps = zpsum.tile([P, D], F32, tag="zps")
                    for i, jj in enumerate(blist):
                        # banks 0..2 come first in the group layout (DVE-S);
                        # bank HOSTK=3 blocks use the host-S compact stream
                        if jj < gnbd:
                            s_ap = sw[:, jj, :]
                        else:
                            s_ap = sh_t[:, loh + (jj - gnbd), :]
                        nc.tensor.matmul(out=z_ps[:], lhsT=s_ap,
                                         rhs=xg[:, jj, :],
                                         start=(i == 0),
                                         stop=(i == len(blist) - 1))

                    z_sb = epool.tile([P, D], F16, tag="zsb")
                    nc.scalar.copy(z_sb[:], z_ps[:])
                    o_ps = opsum.tile([P, D], F32, tag="ops")
                    for ch in range(2):
                        zt_ps = tpsum.tile([P, P], F16, tag="ztps")
                        nc.tensor.transpose(zt_ps[:],
                                            z_sb[:, ch * P:(ch + 1) * P],
                                            ident_t[:])
                        zt_sb = epool.tile([P, P], F16, tag="ztsb")
                        nc.scalar.copy(zt_sb[:], zt_ps[:])
                        nc.tensor.matmul(out=o_ps[:], lhsT=zt_sb[:],
                                         rhs=w_t[:, ch, :],
                                         start=(ch == 0), stop=(ch == 1))
                    o_sb = epool.tile([P, D], F32, tag="osb")
                    nc.vector.tensor_add(o_sb[:], o_ps[:], bias_t[:])
                    nc.sync.dma_start(out_ap[t * P:(t + 1) * P, :], o_sb[:])
                bo += gnb
                bod += gnbd
                boh += int(nb_gh[g])
    nc.compile()
    return nc


def kernel(x, edge_row, edge_col, edge_val, weight, b):
    global _last_results
    assert x.shape == (N_NODES, D)

    nb_cell, cores = _build_structure(
        np.asarray(edge_row), np.asarray(edge_col), np.asarray(edge_val))
    nc = _build_program(nb_cell)

    x16 = np.asarray(x, np.float32).astype(np.float16)
    banks = [np.ascontiguousarray(x16[k * BS:(k + 1) * BS])
             for k in range(NBANK)]
    w16 = np.asarray(weight, np.float32).astype(np.float16)
    bias = np.broadcast_to(
        np.asarray(b, np.float32)[None, :], (P, D)).copy()
    iota = np.tile(np.arange(P, dtype=np.float16)[None, :], (P, 1))
    ident = np.eye(P, dtype=np.float16)

    in_maps = []
    for c in range(NC):
        m = {f"xb{k}": banks[k] for k in range(NBANK)}
        m.update(idxp=cores[c]["idxp"], dv=cores[c]["dv"], sh=cores[c]["sh"],
                 w=w16, bias=bias, iota=iota, ident=ident)
        in_maps.append(m)

    trace = bool(os.environ.get("KERNEL_TRACE"))
    res = run_bass_kernel_spmd(nc, in_maps, list(range(NC)), trace=trace)
    _last_results = res

    out = np.concatenate([res.results[c]["out"][:SH] for c in range(NC)],
                         axis=0)
    return out.astype(np.float32)
